# revision 21
# baseline (speedup 1.0000x reference)
"""Trainium2 Bass kernel for nn_Decoder (30-step scan of a tiny transformer block).

Data-parallel over batch: 32768 rows -> 8 cores x 4096. Feature-major layout
(features on SBUF partitions, batch on free dim), batch tiled by 512 columns.

Host-side algebra removes most per-step work:
  - seq_len==1 attention collapses to A = I + Wo@Wv; x and attn are affine in
    the step inputs, so  r1 = A@x = C + W7@z_t  where C is a per-sample
    constant computed ONCE (host GEMM) and z_t = [state; plan*gate; gate] is 7
    rows stacked in one tile.
  - LN1 (beta1==0) never needs its rstd: relu is positively homogeneous and
    LN2 is scale invariant, so only the *centered* pre-LN1 activation matters.
    Centering is linear -> folded into C / W7 on the host. LN1 costs nothing.
  - The W1 matmul splits the same way: W1@r1c = P1 + U1@z_t with P1 (per-sample
    constant) precomputed on the host and injected into PSUM via an
    identity-weight matmul; U1 is [FF, 7]. The only full-width matmul left per
    step is W2 (fp8 DoubleRow) and the tiny decoder head.
  - LN2's normalize folds into the decoder head: pre1 = (Wd1g@w - m2*rd)
    * rsqrt(var2+eps) on [64, N] instead of normalizing [384, N].
  - g1/g2 folded into host-side weights; zero biases asserted/folded.

fp8e4 is used for P1/h1/W2 (prescaled by 16 to dodge fp8 subnormals; undone in
the residual add, which re-accumulates the low-rank term in f32 PSUM so the
residual path carries no fp8 noise). elu(x) = max(exp(min(x,0))-1, x).
"""

import numpy as np
from contextlib import ExitStack

B, T, D, FF, HID = 32768, 30, 384, 1024, 64
LN_EPS = 1e-5
NCORES = 8
BL = B // NCORES   # 4096 rows per core
TN = 512           # batch tile (one PSUM bank of fp32)
NT = BL // TN      # 8 groups per core
KD = D // 128      # 3 feature chunks
KF = FF // 128     # 8 FF chunks
S1 = 16.0          # fp8 prescale for the W1 path (P1/U1)
S2 = 16.0          # fp8 prescale for W2

_STATE = {}


def _build_nc(t_steps=T, bl=BL):
    import concourse.bass as bass
    import concourse.bacc as bacc
    import concourse.mybir as mybir
    import concourse.tile as tile

    f32 = mybir.dt.float32
    f32r = mybir.dt.float32r
    bf16 = mybir.dt.bfloat16
    fp8 = mybir.dt.float8e4
    AF = mybir.ActivationFunctionType
    OP = mybir.AluOpType
    DR = mybir.MatmulPerfMode.DoubleRow

    nc = bacc.Bacc(trn_type="TRN2", target_bir_lowering=False, debug=False)

    # ---- DRAM tensors ----
    d_plan = nc.dram_tensor("planTg", [t_steps, 4, bl], f32r, kind="ExternalInput").ap()
    d_st0 = nc.dram_tensor("state0T", [3, bl], f32r, kind="ExternalInput").ap()
    d_c0c = nc.dram_tensor("c0cT", [D, bl], bf16, kind="ExternalInput").ap()
    d_p1 = nc.dram_tensor("p1T", [FF, bl], fp8, kind="ExternalInput").ap()
    d_w7cs = nc.dram_tensor("w7cs", [7, D], f32r, kind="ExternalInput").ap()
    d_u1 = nc.dram_tensor("u1t", [7, FF], f32r, kind="ExternalInput").ap()
    d_w2 = nc.dram_tensor("w2s8", [128, KF, D], fp8, kind="ExternalInput").ap()
    d_wd1 = nc.dram_tensor("wd1t", [128, KD, HID], bf16, kind="ExternalInput").ap()
    d_wd2 = nc.dram_tensor("wd2t", [HID, 3], bf16, kind="ExternalInput").ap()
    d_rdneg = nc.dram_tensor("rdnegr", [1, HID], bf16, kind="ExternalInput").ap()
    d_bd2 = nc.dram_tensor("bd2v", [3, 1], f32, kind="ExternalInput").ap()
    d_ones = nc.dram_tensor("ones64", [128, HID], bf16, kind="ExternalInput").ap()
    d_eye = nc.dram_tensor("eye128", [128, 128], fp8, kind="ExternalInput").ap()
    d_out = nc.dram_tensor("outT", [t_steps, 3, bl], f32r, kind="ExternalOutput").ap()

    with tile.TileContext(nc) as tc, ExitStack() as ctx:
        wp = ctx.enter_context(tc.tile_pool(name="w", bufs=1))

        def wtile(name, shape, src, dt_):
            t_ = wp.tile(shape, dt_, tag=name, name=name)
            nc.sync.dma_start(t_[:], src)
            return t_

        w7cs = wtile("w7cs", [7, D], d_w7cs[:, :], f32r)
        u1t = wtile("u1t", [7, FF], d_u1[:, :], f32r)
        w2t = wtile("w2t", [128, KF, D], d_w2[:, :, :], fp8)
        wd1t = wtile("wd1t", [128, KD, HID], d_wd1[:, :, :], bf16)
        wd2t = wtile("wd2t", [HID, 3], d_wd2[:, :], bf16)
        rdnegr = wtile("rdnegr", [1, HID], d_rdneg[:, :], bf16)
        bd2v = wtile("bd2v", [3, 1], d_bd2[:, :], f32)
        ones64 = wtile("ones64", [128, HID], d_ones[:, :], bf16)
        eye128 = wtile("eye128", [128, 128], d_eye[:, :], fp8)
        epsb = wp.tile([HID, 1], f32, tag="epsb", name="epsb")
        nc.vector.memset(epsb[:], LN_EPS)

        # per-group persistent activation tiles (in-place rewrite per step)
        c0c_g, p1_g, h8_g, w3_g = [], [], [], []
        for n in range(NT):
            cs = slice(n * TN, (n + 1) * TN)
            c = wp.tile([128, KD, TN], bf16, tag=f"c0c{n}", name=f"c0c{n}")
            for k in range(KD):
                nc.sync.dma_start(c[:, k, :], d_c0c[k * 128:(k + 1) * 128, cs])
            c0c_g.append(c)
            p = wp.tile([128, KF, TN], fp8, tag=f"p1{n}", name=f"p1{n}")
            for q in range(KF):
                nc.sync.dma_start(p[:, q, :], d_p1[q * 128:(q + 1) * 128, cs])
            p1_g.append(p)
            h8_g.append(wp.tile([128, KF, TN], fp8, tag=f"h8{n}", name=f"h8{n}"))
            w3_g.append(wp.tile([128, KD, TN], bf16, tag=f"w3{n}", name=f"w3{n}"))

        # state+plan tiles: z7[t] rows 0..2 = state_t, rows 3..6 = [plan_t*gate; gate]
        zp = ctx.enter_context(tc.tile_pool(name="zp", bufs=2))
        z7s = [zp.tile([7, bl], f32r, tag="z7", name=f"z7_{t}") for t in range(t_steps + 1)]
        nc.sync.dma_start(z7s[0][0:3, :], d_st0[:, :])
        for t in range(t_steps):
            nc.sync.dma_start(z7s[t][3:7, :], d_plan[t, :, :])

        # working pools
        sp = ctx.enter_context(tc.tile_pool(name="sp", bufs=2))
        pp = ctx.enter_context(tc.tile_pool(name="pp", bufs=8, space="PSUM"))

        def w1_phase(t, n):
            # h1 = relu(U1@z7 + P1) -> fp8 (scaled by S1)
            cs = slice(n * TN, (n + 1) * TN)
            h8 = h8_g[n]
            for q in range(KF):
                qs = slice(q * 128, (q + 1) * 128)
                ps = pp.tile([128, TN], f32, tag="ps", name="ps")
                nc.tensor.matmul(ps[:], u1t[:, qs], z7s[t][:, cs],
                                 start=True, stop=False)
                nc.tensor.matmul(ps[:], eye128[:, :], p1_g[n][:, q, :],
                                 start=False, stop=True)
                if q % 2 == 0:
                    nc.scalar.activation(h8[:, q, :], ps[:], AF.Relu)
                else:
                    nc.vector.tensor_scalar(h8[:, q, :], ps[:], 0.0, None, OP.max)

        def w2_phase(t, n):
            cs = slice(n * TN, (n + 1) * TN)
            h8 = h8_g[n]
            w3 = w3_g[n]
            # w = (W7cs@z7 + W2s@h1)/(S1*S2) + C0c -> bf16   (low-rank residual
            # term re-accumulated in f32 psum: no fp8 noise on the residual)
            for m in range(KD):
                ms = slice(m * 128, (m + 1) * 128)
                ps = pp.tile([128, TN], f32, tag="ps", name="ps")
                nc.tensor.matmul(ps[:], w7cs[:, ms], z7s[t][:, cs],
                                 start=True, stop=False)
                for kk in range(KF // 2):
                    nc.tensor.matmul(ps[:], w2t[:, 2 * kk:2 * kk + 2, ms],
                                     h8[:, 2 * kk:2 * kk + 2, :],
                                     start=False, stop=(kk == KF // 2 - 1),
                                     perf_mode=DR)
                nc.vector.scalar_tensor_tensor(w3[:, m, :], ps[:], 1.0 / (S1 * S2),
                                               c0c_g[n][:, m, :], OP.mult, OP.add)

            # LN2 stats: m2 = sum(w)/D, var = sum(w^2)/D - m2^2
            mps = pp.tile([HID, TN], f32, tag="ps", name="ps")
            for k in range(KD):
                nc.tensor.matmul(mps[:], ones64[:, :], w3[:, k, :],
                                 start=(k == 0), stop=(k == KD - 1))
            m2sb = sp.tile([HID, TN], bf16, tag="m2", name="m2")
            nc.scalar.activation(m2sb[:], mps[:], AF.Copy, scale=1.0 / D)

            # dps = Wd1g@w - m2*rd, with the m2*rd correction folded in as a
            # K=1 matmul (rdneg stationary, m2 row moving)
            dps = pp.tile([HID, TN], f32, tag="ps", name="ps")
            for k in range(KD):
                nc.tensor.matmul(dps[:], wd1t[:, k, :], w3[:, k, :],
                                 start=(k == 0), stop=False)
            nc.tensor.matmul(dps[:], rdnegr[:, :], m2sb[0:1, :],
                             start=False, stop=True)

            wsq = sp.tile([128, KD, TN], bf16, tag="wsq", name="wsq")
            nc.scalar.activation(wsq[:, :, :], w3[:, :, :], AF.Square)
            eps2 = pp.tile([HID, TN], f32, tag="ps", name="ps")
            for k in range(KD):
                nc.tensor.matmul(eps2[:], ones64[:, :], wsq[:, k, :],
                                 start=(k == 0), stop=(k == KD - 1))
            ew2 = sp.tile([HID, TN], bf16, tag="ew2", name="ew2")
            nc.scalar.activation(ew2[:], eps2[:], AF.Copy, scale=1.0 / D)

            # inv2 = rsqrt(var+eps);  pre1 = dps * inv2
            msq = sp.tile([HID, TN], bf16, tag="msq", name="msq")
            nc.vector.tensor_tensor(msq[:], m2sb[:], m2sb[:], OP.mult)
            varb = sp.tile([HID, TN], bf16, tag="varb", name="varb")
            nc.vector.tensor_tensor(varb[:], ew2[:], msq[:], OP.subtract)
            lnv = sp.tile([HID, TN], bf16, tag="lnv", name="lnv")
            nc.scalar.activation(lnv[:], varb[:], AF.Ln, bias=epsb[:])
            inv2 = sp.tile([HID, TN], bf16, tag="inv2", name="inv2")
            nc.scalar.activation(inv2[:], lnv[:], AF.Exp, scale=-0.5)
            pre1 = sp.tile([HID, TN], bf16, tag="pre1", name="pre1")
            nc.vector.tensor_tensor(pre1[:], dps[:], inv2[:], OP.mult)

            # elu = max(exp(min(x,0))-1, x)
            emin = sp.tile([HID, TN], bf16, tag="emin", name="emin")
            nc.vector.tensor_scalar(emin[:], pre1[:], 0.0, None, OP.min)
            eexp = sp.tile([HID, TN], bf16, tag="eexp", name="eexp")
            nc.scalar.activation(eexp[:], emin[:], AF.Exp)
            el = sp.tile([HID, TN], bf16, tag="el", name="el", bufs=NT)
            nc.vector.scalar_tensor_tensor(el[:], eexp[:], 1.0, pre1[:],
                                           OP.subtract, OP.max)
            return el

        def tail_phase(t, n, el):
            # upd = Wd2 @ elu + bd2 ; state_{t+1} = state_t + upd
            cs = slice(n * TN, (n + 1) * TN)
            d2 = pp.tile([3, TN], f32, tag="ps", name="ps")
            nc.tensor.matmul(d2[:], wd2t[:, :], el[:], start=True, stop=True)
            nc.vector.scalar_tensor_tensor(z7s[t + 1][0:3, cs], d2[:], bd2v[:],
                                           z7s[t][0:3, cs], OP.add, OP.add)

        # 3-stage software pipeline, skewed by group: W1(g) || W2+stats(g-1)
        # || tail(g-2).  Keeps the PE instruction stream free of long waits
        # (no el-dependent matmul is ever followed by independent work).
        els = {}
        for t in range(t_steps):
            for n in range(NT + 2):
                if n < NT:
                    w1_phase(t, n)
                if 1 <= n <= NT:
                    els[n - 1] = w2_phase(t, n - 1)
                if n >= 2:
                    tail_phase(t, n - 2, els.pop(n - 2))
            nc.sync.dma_start(d_out[t, :, :], z7s[t + 1][0:3, :])

    import concourse.bacc as bacc_mod
    if not getattr(bacc_mod, "_act_tables_patched", False):
        _orig_tables = bacc_mod.get_activation_tables
        _KEEP = "natural_log_exp_and_others"

        def _one_set_tables(arch):
            t = _orig_tables(arch)
            return {name: (fns if name == _KEEP else set()) for name, fns in t.items()}

        bacc_mod.get_activation_tables = _one_set_tables
        bacc_mod._act_tables_patched = True
    nc.compile()
    return nc


def _prep(inputs):
    """Host-side: fold the attention block, LN1, gains and biases into
    C0c/P1/W7c/U1; transpose weights to lhsT layouts; shard batch."""
    import ml_dtypes

    g = {k: np.asarray(v, dtype=np.float32) for k, v in inputs.items()}
    for zk in ("beta1", "b1", "b2", "beta2", "bd1"):
        assert np.max(np.abs(g[zk])) == 0.0, f"kernel assumes {zk} == 0"

    Wv = g["Wqkv"][2 * D:, :]
    bv = g["bqkv"][2 * D:]
    A = np.eye(D, dtype=np.float32) + g["Wo"] @ Wv           # [D, D]
    ab = g["Wo"] @ bv + g["bo"]                              # [D]

    iH = g["init_hidden"] + g["bs"][None, :]                 # [B, D]
    Cfull = iH @ A.T + ab[None, :]                           # [B, D] (host GEMM)
    C0cf = (Cfull - Cfull.mean(axis=1, keepdims=True)) * g["g1"][None, :]

    # W7 rows match z7 rows: [state (3); plan*gate (3); gate (1)]
    W7 = np.concatenate([(A @ g["Ws"]).T, (A @ g["Wp"]).T, (A @ g["bp"])[None, :]], 0)
    W7c = (W7 - W7.mean(axis=1, keepdims=True)) * g["g1"][None, :]  # [7, D]

    U1 = S1 * (g["W1"] @ W7c.T)                              # [FF, 7]
    P1 = S1 * (C0cf @ g["W1"].T)                             # [B, FF] (host GEMM)

    b16 = lambda a: np.ascontiguousarray(a).astype(ml_dtypes.bfloat16)
    f8 = lambda a: np.ascontiguousarray(np.clip(a, -240, 240)).astype(ml_dtypes.float8_e4m3)

    def lhsT_pack(w, kchunks):   # w: [out, in] -> [128, kchunks, out]
        return w.T.reshape(kchunks, 128, w.shape[0]).transpose(1, 0, 2)

    Wd1g = (g["Wd1"] * g["g2"][None, :]).astype(ml_dtypes.bfloat16).astype(np.float32)
    shared = {
        "w7cs": np.ascontiguousarray(W7c * (S1 * S2)),
        "u1t": np.ascontiguousarray(U1.T),
        "w2s8": f8(lhsT_pack(g["W2"] * S2, KF)),
        "wd1t": b16(lhsT_pack(Wd1g, KD)),
        "wd2t": b16(g["Wd2"].T),
        "rdnegr": b16(-Wd1g.sum(axis=1).reshape(1, -1)),
        "bd2v": np.ascontiguousarray(g["bd2"].reshape(-1, 1)),
        "ones64": np.ones((128, HID), dtype=ml_dtypes.bfloat16),
        "eye128": np.eye(128, dtype=np.float32).astype(ml_dtypes.float8_e4m3),
    }

    gate = g["gate"][:, 0]                                   # [B]
    pgate = g["plan"] * g["gate"][:, None, :]                # [B, T, 3]
    planT = pgate.transpose(1, 2, 0)                         # [T, 3, B]
    planTg = np.concatenate(
        [planT, np.broadcast_to(gate[None, None, :], (T, 1, B))], axis=1
    )                                                        # [T, 4, B]
    st0 = g["init_state"][:, :3].T                           # [3, B]
    c0cT = C0cf.T.astype(ml_dtypes.bfloat16)                 # [D, B]
    p1T = np.clip(P1.T, -240, 240).astype(ml_dtypes.float8_e4m3)  # [FF, B]

    in_maps = []
    for c in range(NCORES):
        cs = slice(c * BL, (c + 1) * BL)
        m = dict(shared)
        m["c0cT"] = np.ascontiguousarray(c0cT[:, cs])
        m["p1T"] = np.ascontiguousarray(p1T[:, cs])
        m["planTg"] = np.ascontiguousarray(planTg[:, :, cs])
        m["state0T"] = np.ascontiguousarray(st0[:, cs])
        in_maps.append(m)
    return in_maps


def run(inputs, trace=False, trace_kwargs=None):
    from concourse.bass_utils import run_bass_kernel_spmd

    if "nc" not in _STATE:
        _STATE["nc"] = _build_nc()
    in_maps = _prep(inputs)
    res = run_bass_kernel_spmd(
        _STATE["nc"], in_maps, list(range(NCORES)), trace=trace,
        **(trace_kwargs or {}),
    )
    out = np.empty((B, T, 3), dtype=np.float32)
    for c in range(NCORES):
        outT = np.asarray(res.results[c]["outT"], dtype=np.float32)  # [T, 3, BL]
        out[c * BL:(c + 1) * BL] = outT.transpose(2, 0, 1)
    return out, res


def kernel(**inputs) -> np.ndarray:
    out, _ = run(inputs)
    return out


# revision 27
# speedup vs baseline: 1.9358x; 1.9358x over previous
"""Trainium2 Bass kernel for nn_Decoder (30-step scan of a tiny transformer block).

Data-parallel over batch: 32768 rows -> 8 cores x 4096. Feature-major layout
(features on SBUF partitions, batch on free dim), batch tiled by 512 columns.

Host-side algebra removes most per-step work:
  - seq_len==1 attention collapses to A = I + Wo@Wv; x and attn are affine in
    the step inputs, so  r1 = A@x = C + W7@z_t  where C is a per-sample
    constant computed ONCE (host GEMM) and z_t = [state; plan*gate; gate] is 7
    rows stacked in one tile.
  - LN1 (beta1==0) never needs its rstd: relu is positively homogeneous and
    LN2 is scale invariant, so only the *centered* pre-LN1 activation matters.
    Centering is linear -> folded into C / W7 on the host. LN1 costs nothing.
  - The W1 matmul splits the same way: W1@r1c = P1 + U1@z_t with P1 (per-sample
    constant) precomputed on the host and injected into PSUM via an
    identity-weight matmul; U1 is [FF, 7]. The only full-width matmul left per
    step is W2 (fp8 DoubleRow) and the tiny decoder head.
  - LN2's normalize folds into the decoder head: pre1 = (Wd1g@w - m2*rd)
    * rsqrt(var2+eps) on [64, N] instead of normalizing [384, N].
  - g1/g2 folded into host-side weights; zero biases asserted/folded.

fp8e4 is used for P1/h1/W2 (prescaled by 16 to dodge fp8 subnormals; undone in
the residual add, which re-accumulates the low-rank term in f32 PSUM so the
residual path carries no fp8 noise). elu(x) = max(exp(min(x,0))-1, x).
"""

import numpy as np
from contextlib import ExitStack

B, T, D, FF, HID = 32768, 30, 384, 1024, 64
LN_EPS = 1e-5
NCORES = 8
BL = B // NCORES   # 4096 rows per core
TN = 512           # batch tile (one PSUM bank of fp32)
NT = BL // TN      # 8 groups per core
KD = D // 128      # 3 feature chunks
KF = FF // 128     # 8 FF chunks
S1 = 16.0          # fp8 prescale for the W1 path (P1/U1)
S2 = 16.0          # fp8 prescale for W2

_STATE = {}


def _build_nc(t_steps=T, bl=BL):
    import concourse.bass as bass
    import concourse.bacc as bacc
    import concourse.mybir as mybir
    import concourse.tile as tile

    f32 = mybir.dt.float32
    f32r = mybir.dt.float32r
    bf16 = mybir.dt.bfloat16
    fp8 = mybir.dt.float8e4
    AF = mybir.ActivationFunctionType
    OP = mybir.AluOpType
    DR = mybir.MatmulPerfMode.DoubleRow

    nc = bacc.Bacc(trn_type="TRN2", target_bir_lowering=False, debug=False)

    # ---- DRAM tensors ----
    d_plan = nc.dram_tensor("planTg", [t_steps, 4, bl], f32r, kind="ExternalInput").ap()
    d_st0 = nc.dram_tensor("state0T", [3, bl], f32r, kind="ExternalInput").ap()
    d_c0c = nc.dram_tensor("c0cT", [D, bl], bf16, kind="ExternalInput").ap()
    d_p1 = nc.dram_tensor("p1T", [FF, bl], fp8, kind="ExternalInput").ap()
    d_w7cs = nc.dram_tensor("w7cs", [128, D], f32r, kind="ExternalInput").ap()
    d_u1 = nc.dram_tensor("u1t", [128, FF], f32r, kind="ExternalInput").ap()
    d_w2 = nc.dram_tensor("w2s8", [128, KF, D], fp8, kind="ExternalInput").ap()
    d_wd1 = nc.dram_tensor("wd1t", [128, KD, HID], bf16, kind="ExternalInput").ap()
    d_wd2 = nc.dram_tensor("wd2t", [HID, 3], bf16, kind="ExternalInput").ap()
    d_rdneg = nc.dram_tensor("rdnegr", [HID, HID], bf16, kind="ExternalInput").ap()
    d_bd2 = nc.dram_tensor("bd2v", [3, 1], f32, kind="ExternalInput").ap()
    d_ones = nc.dram_tensor("ones64", [128, HID], bf16, kind="ExternalInput").ap()
    d_eye = nc.dram_tensor("eye128", [128, 128], fp8, kind="ExternalInput").ap()
    d_zero = nc.dram_tensor("zeros121", [121, bl], f32r, kind="ExternalInput").ap()
    d_out = nc.dram_tensor("outT", [t_steps, 3, bl], f32r, kind="ExternalOutput").ap()

    with tile.TileContext(nc) as tc, ExitStack() as ctx:
        wp = ctx.enter_context(tc.tile_pool(name="w", bufs=1))

        def wtile(name, shape, src, dt_):
            t_ = wp.tile(shape, dt_, tag=name, name=name)
            nc.sync.dma_start(t_[:], src)
            return t_

        w7cs = wtile("w7cs", [128, D], d_w7cs[:, :], f32r)
        u1t = wtile("u1t", [128, FF], d_u1[:, :], f32r)
        w2t = wtile("w2t", [128, KF, D], d_w2[:, :, :], fp8)
        wd1t = wtile("wd1t", [128, KD, HID], d_wd1[:, :, :], bf16)
        wd2t = wtile("wd2t", [HID, 3], d_wd2[:, :], bf16)
        rdnegr = wtile("rdnegr", [HID, HID], d_rdneg[:, :], bf16)
        bd2v = wtile("bd2v", [3, 1], d_bd2[:, :], f32)
        ones64 = wtile("ones64", [128, HID], d_ones[:, :], bf16)
        eye128 = wtile("eye128", [128, 128], d_eye[:, :], fp8)
        epsb = wp.tile([HID, 1], f32, tag="epsb", name="epsb")
        nc.vector.memset(epsb[:], LN_EPS)

        # per-group persistent activation tiles (in-place rewrite per step)
        c0c_g, p1_g, h8_g, w3_g = [], [], [], []
        for n in range(NT):
            cs = slice(n * TN, (n + 1) * TN)
            c = wp.tile([128, KD, TN], bf16, tag=f"c0c{n}", name=f"c0c{n}")
            for k in range(KD):
                nc.sync.dma_start(c[:, k, :], d_c0c[k * 128:(k + 1) * 128, cs])
            c0c_g.append(c)
            p = wp.tile([128, KF, TN], fp8, tag=f"p1{n}", name=f"p1{n}")
            for q in range(KF):
                nc.sync.dma_start(p[:, q, :], d_p1[q * 128:(q + 1) * 128, cs])
            p1_g.append(p)
            h8_g.append(wp.tile([128, KF, TN], fp8, tag=f"h8{n}", name=f"h8{n}"))
            w3_g.append(wp.tile([128, KD, TN], bf16, tag=f"w3{n}", name=f"w3{n}"))

        # state+plan tiles: z7[t] rows 0..2 = state_t, rows 3..6 = [plan_t*gate; gate]
        zp = ctx.enter_context(tc.tile_pool(name="zp", bufs=2))
        z7s = [zp.tile([128, bl], f32r, tag="z7", name=f"z7_{t}") for t in range(t_steps + 1)]
        # rows 7:128 are a K-pad read by the (zero-padded) U1/W7cs matmuls;
        # zero both physical ring slots once so they never contain Inf/NaN bits
        nc.sync.dma_start(z7s[0][7:128, :], d_zero[:, :])
        nc.sync.dma_start(z7s[1][7:128, :], d_zero[:, :])
        nc.sync.dma_start(z7s[0][0:3, :], d_st0[:, :])
        for t in range(t_steps):
            nc.sync.dma_start(z7s[t][3:7, :], d_plan[t, :, :])

        # working pools
        sp = ctx.enter_context(tc.tile_pool(name="sp", bufs=2))
        pp = ctx.enter_context(tc.tile_pool(name="pp", bufs=8, space="PSUM"))

        def w1_phase(t, n):
            # h1 = relu(U1@z7 + P1) -> fp8 (scaled by S1)
            cs = slice(n * TN, (n + 1) * TN)
            h8 = h8_g[n]
            for q in range(KF):
                qs = slice(q * 128, (q + 1) * 128)
                ps = pp.tile([128, TN], f32, tag="ps", name="ps")
                nc.tensor.matmul(ps[:], u1t[:, qs], z7s[t][:, cs],
                                 start=True, stop=False)
                nc.tensor.matmul(ps[:], eye128[:, :], p1_g[n][:, q, :],
                                 start=False, stop=True)
                if q % 2 == 0:
                    nc.scalar.activation(h8[:, q, :], ps[:], AF.Relu)
                else:
                    nc.vector.tensor_scalar(h8[:, q, :], ps[:], 0.0, None, OP.max)

        def w2_phase(t, n):
            cs = slice(n * TN, (n + 1) * TN)
            h8 = h8_g[n]
            w3 = w3_g[n]
            # w = (W7cs@z7 + W2s@h1)/(S1*S2) + C0c -> bf16   (low-rank residual
            # term re-accumulated in f32 psum: no fp8 noise on the residual)
            for m in range(KD):
                ms = slice(m * 128, (m + 1) * 128)
                ps = pp.tile([128, TN], f32, tag="ps", name="ps")
                nc.tensor.matmul(ps[:], w7cs[:, ms], z7s[t][:, cs],
                                 start=True, stop=False)
                for kk in range(KF // 2):
                    nc.tensor.matmul(ps[:], w2t[:, 2 * kk:2 * kk + 2, ms],
                                     h8[:, 2 * kk:2 * kk + 2, :],
                                     start=False, stop=(kk == KF // 2 - 1),
                                     perf_mode=DR)
                nc.vector.scalar_tensor_tensor(w3[:, m, :], ps[:], 1.0 / (S1 * S2),
                                               c0c_g[n][:, m, :], OP.mult, OP.add)

        def stats_phase(t, n):
            w3 = w3_g[n]
            # LN2 stats: m2 = sum(w)/D, var = sum(w^2)/D - m2^2
            mps = pp.tile([HID, TN], f32, tag="ps", name="ps")
            for k in range(KD):
                nc.tensor.matmul(mps[:], ones64[:, :], w3[:, k, :],
                                 start=(k == 0), stop=(k == KD - 1))
            m2sb = sp.tile([HID, TN], bf16, tag="m2", name="m2")
            nc.scalar.activation(m2sb[:], mps[:], AF.Copy, scale=1.0 / D)

            # dps = Wd1g@w - m2*rd, with the m2*rd correction folded in as a
            # K=1 matmul (rdneg stationary, m2 row moving)
            dps = pp.tile([HID, TN], f32, tag="ps", name="ps")
            for k in range(KD):
                nc.tensor.matmul(dps[:], wd1t[:, k, :], w3[:, k, :],
                                 start=(k == 0), stop=False)
            nc.tensor.matmul(dps[:], rdnegr[:, :], m2sb[:, :],
                             start=False, stop=True)

            wsq = sp.tile([128, KD, TN], bf16, tag="wsq", name="wsq")
            nc.scalar.activation(wsq[:, :, :], w3[:, :, :], AF.Square)
            eps2 = pp.tile([HID, TN], f32, tag="ps", name="ps")
            for k in range(KD):
                nc.tensor.matmul(eps2[:], ones64[:, :], wsq[:, k, :],
                                 start=(k == 0), stop=(k == KD - 1))
            ew2 = sp.tile([HID, TN], bf16, tag="ew2", name="ew2")
            nc.scalar.activation(ew2[:], eps2[:], AF.Copy, scale=1.0 / D)

            # inv2 = rsqrt(var+eps);  pre1 = dps * inv2
            msq = sp.tile([HID, TN], bf16, tag="msq", name="msq")
            nc.vector.tensor_tensor(msq[:], m2sb[:], m2sb[:], OP.mult)
            varb = sp.tile([HID, TN], bf16, tag="varb", name="varb")
            nc.vector.tensor_tensor(varb[:], ew2[:], msq[:], OP.subtract)
            lnv = sp.tile([HID, TN], bf16, tag="lnv", name="lnv")
            nc.scalar.activation(lnv[:], varb[:], AF.Ln, bias=epsb[:])
            inv2 = sp.tile([HID, TN], bf16, tag="inv2", name="inv2")
            nc.scalar.activation(inv2[:], lnv[:], AF.Exp, scale=-0.5)
            pre1 = sp.tile([HID, TN], bf16, tag="pre1", name="pre1")
            nc.vector.tensor_tensor(pre1[:], dps[:], inv2[:], OP.mult)

            # elu = max(exp(min(x,0))-1, x)
            emin = sp.tile([HID, TN], bf16, tag="emin", name="emin")
            nc.vector.tensor_scalar(emin[:], pre1[:], 0.0, None, OP.min)
            eexp = sp.tile([HID, TN], bf16, tag="eexp", name="eexp")
            nc.scalar.activation(eexp[:], emin[:], AF.Exp)
            el = sp.tile([HID, TN], bf16, tag="el", name="el", bufs=NT)
            nc.vector.scalar_tensor_tensor(el[:], eexp[:], 1.0, pre1[:],
                                           OP.subtract, OP.max)
            return el

        def tail_phase(t, n, el):
            # upd = Wd2 @ elu + bd2 ; state_{t+1} = state_t + upd
            cs = slice(n * TN, (n + 1) * TN)
            d2 = pp.tile([3, TN], f32, tag="ps", name="ps")
            nc.tensor.matmul(d2[:], wd2t[:, :], el[:], start=True, stop=True)
            nc.vector.scalar_tensor_tensor(z7s[t + 1][0:3, cs], d2[:], bd2v[:],
                                           z7s[t][0:3, cs], OP.add, OP.add)

        # 4-stage software pipeline over global slots s = t*NT + n, skewed by
        # one group per stage: W1(s) || W2(s-1) || stats+normalize(s-2) ||
        # decoder-tail(s-3). Slots cross step boundaries (state of group g is
        # ready 5 slots before step t+1 group g needs it). Keeps the PE
        # instruction stream free of long waits and HAM warm.
        els = {}
        S = t_steps * NT
        for s in range(S + 3):
            if s < S:
                w1_phase(s // NT, s % NT)
            if 1 <= s < S + 1:
                w2_phase((s - 1) // NT, (s - 1) % NT)
            if 2 <= s < S + 2:
                els[s - 2] = stats_phase((s - 2) // NT, (s - 2) % NT)
            if 3 <= s < S + 3:
                t3, n3 = (s - 3) // NT, (s - 3) % NT
                tail_phase(t3, n3, els.pop(s - 3))
                if n3 == NT - 1:
                    nc.sync.dma_start(d_out[t3, :, :], z7s[t3 + 1][0:3, :])

    import concourse.bacc as bacc_mod
    if not getattr(bacc_mod, "_act_tables_patched", False):
        _orig_tables = bacc_mod.get_activation_tables
        _KEEP = "natural_log_exp_and_others"

        def _one_set_tables(arch):
            t = _orig_tables(arch)
            return {name: (fns if name == _KEEP else set()) for name, fns in t.items()}

        bacc_mod.get_activation_tables = _one_set_tables
        bacc_mod._act_tables_patched = True
    nc.compile()
    return nc


def _prep(inputs):
    """Host-side: fold the attention block, LN1, gains and biases into
    C0c/P1/W7c/U1; transpose weights to lhsT layouts; shard batch."""
    import ml_dtypes

    g = {k: np.asarray(v, dtype=np.float32) for k, v in inputs.items()}
    for zk in ("beta1", "b1", "b2", "beta2", "bd1"):
        assert np.max(np.abs(g[zk])) == 0.0, f"kernel assumes {zk} == 0"

    Wv = g["Wqkv"][2 * D:, :]
    bv = g["bqkv"][2 * D:]
    A = np.eye(D, dtype=np.float32) + g["Wo"] @ Wv           # [D, D]
    ab = g["Wo"] @ bv + g["bo"]                              # [D]

    iH = g["init_hidden"] + g["bs"][None, :]                 # [B, D]
    Cfull = iH @ A.T + ab[None, :]                           # [B, D] (host GEMM)
    C0cf = (Cfull - Cfull.mean(axis=1, keepdims=True)) * g["g1"][None, :]

    # W7 rows match z7 rows: [state (3); plan*gate (3); gate (1)]
    W7 = np.concatenate([(A @ g["Ws"]).T, (A @ g["Wp"]).T, (A @ g["bp"])[None, :]], 0)
    W7c = (W7 - W7.mean(axis=1, keepdims=True)) * g["g1"][None, :]  # [7, D]

    U1 = S1 * (g["W1"] @ W7c.T)                              # [FF, 7]
    P1 = S1 * (C0cf @ g["W1"].T)                             # [B, FF] (host GEMM)

    b16 = lambda a: np.ascontiguousarray(a).astype(ml_dtypes.bfloat16)
    f8 = lambda a: np.ascontiguousarray(np.clip(a, -240, 240)).astype(ml_dtypes.float8_e4m3)

    def lhsT_pack(w, kchunks):   # w: [out, in] -> [128, kchunks, out]
        return w.T.reshape(kchunks, 128, w.shape[0]).transpose(1, 0, 2)

    Wd1g = (g["Wd1"] * g["g2"][None, :]).astype(ml_dtypes.bfloat16).astype(np.float32)
    pad128 = lambda a: np.concatenate(
        [a, np.zeros((128 - a.shape[0], a.shape[1]), np.float32)], 0)
    shared = {
        "w7cs": pad128(np.ascontiguousarray(W7c * (S1 * S2))),
        "u1t": pad128(np.ascontiguousarray(U1.T)),
        "w2s8": f8(lhsT_pack(g["W2"] * S2, KF)),
        "wd1t": b16(lhsT_pack(Wd1g, KD)),
        "wd2t": b16(g["Wd2"].T),
        "rdnegr": b16(np.tile(-Wd1g.sum(axis=1).reshape(1, -1) / HID, (HID, 1))),
        "bd2v": np.ascontiguousarray(g["bd2"].reshape(-1, 1)),
        "ones64": np.ones((128, HID), dtype=ml_dtypes.bfloat16),
        "eye128": np.eye(128, dtype=np.float32).astype(ml_dtypes.float8_e4m3),
    }

    gate = g["gate"][:, 0]                                   # [B]
    pgate = g["plan"] * g["gate"][:, None, :]                # [B, T, 3]
    planT = pgate.transpose(1, 2, 0)                         # [T, 3, B]
    planTg = np.concatenate(
        [planT, np.broadcast_to(gate[None, None, :], (T, 1, B))], axis=1
    )                                                        # [T, 4, B]
    st0 = g["init_state"][:, :3].T                           # [3, B]
    c0cT = C0cf.T.astype(ml_dtypes.bfloat16)                 # [D, B]
    p1T = np.clip(P1.T, -240, 240).astype(ml_dtypes.float8_e4m3)  # [FF, B]

    in_maps = []
    for c in range(NCORES):
        cs = slice(c * BL, (c + 1) * BL)
        m = dict(shared)
        m["c0cT"] = np.ascontiguousarray(c0cT[:, cs])
        m["p1T"] = np.ascontiguousarray(p1T[:, cs])
        m["planTg"] = np.ascontiguousarray(planTg[:, :, cs])
        m["zeros121"] = np.zeros((121, BL), np.float32)
        m["state0T"] = np.ascontiguousarray(st0[:, cs])
        in_maps.append(m)
    return in_maps


def run(inputs, trace=False, trace_kwargs=None):
    from concourse.bass_utils import run_bass_kernel_spmd

    if "nc" not in _STATE:
        _STATE["nc"] = _build_nc()
    in_maps = _prep(inputs)
    res = run_bass_kernel_spmd(
        _STATE["nc"], in_maps, list(range(NCORES)), trace=trace,
        **(trace_kwargs or {}),
    )
    out = np.empty((B, T, 3), dtype=np.float32)
    for c in range(NCORES):
        outT = np.asarray(res.results[c]["outT"], dtype=np.float32)  # [T, 3, BL]
        out[c * BL:(c + 1) * BL] = outT.transpose(2, 0, 1)
    return out, res


def kernel(**inputs) -> np.ndarray:
    out, _ = run(inputs)
    return out


# revision 29
# speedup vs baseline: 2.0043x; 1.0354x over previous
"""Trainium2 Bass kernel for nn_Decoder (30-step scan of a tiny transformer block).

Data-parallel over batch: 32768 rows -> 8 cores x 4096. Feature-major layout
(features on SBUF partitions, batch on free dim), batch tiled by 512 columns.

Host-side algebra removes most per-step work:
  - seq_len==1 attention collapses to A = I + Wo@Wv; x and attn are affine in
    the step inputs, so  r1 = A@x = C + W7@z_t  where C is a per-sample
    constant computed ONCE (host GEMM) and z_t = [state; plan*gate; gate] is 7
    rows stacked in one tile.
  - LN1 (beta1==0) never needs its rstd: relu is positively homogeneous and
    LN2 is scale invariant, so only the *centered* pre-LN1 activation matters.
    Centering is linear -> folded into C / W7 on the host. LN1 costs nothing.
  - The W1 matmul splits the same way: W1@r1c = P1 + U1@z_t with P1 (per-sample
    constant) precomputed on the host and injected into PSUM via an
    identity-weight matmul; U1 is [FF, 7]. The only full-width matmul left per
    step is W2 (fp8 DoubleRow) and the tiny decoder head.
  - LN2's normalize folds into the decoder head: pre1 = (Wd1g@w - m2*rd)
    * rsqrt(var2+eps) on [64, N] instead of normalizing [384, N].
  - g1/g2 folded into host-side weights; zero biases asserted/folded.

fp8e4 is used for P1/h1/W2 (prescaled by 16 to dodge fp8 subnormals; undone in
the residual add, which re-accumulates the low-rank term in f32 PSUM so the
residual path carries no fp8 noise). elu(x) = max(exp(min(x,0))-1, x).
"""

import numpy as np
from contextlib import ExitStack

B, T, D, FF, HID = 32768, 30, 384, 1024, 64
LN_EPS = 1e-5
NCORES = 8
BL = B // NCORES   # 4096 rows per core
TN = 512           # batch tile (one PSUM bank of fp32)
NT = BL // TN      # 8 groups per core
KD = D // 128      # 3 feature chunks
KF = FF // 128     # 8 FF chunks
S1 = 16.0          # fp8 prescale for the W1 path (P1/U1)
S2 = 16.0          # fp8 prescale for W2

_STATE = {}


def _build_nc(t_steps=T, bl=BL):
    import concourse.bass as bass
    import concourse.bacc as bacc
    import concourse.mybir as mybir
    import concourse.tile as tile

    f32 = mybir.dt.float32
    f32r = mybir.dt.float32r
    bf16 = mybir.dt.bfloat16
    fp8 = mybir.dt.float8e4
    AF = mybir.ActivationFunctionType
    OP = mybir.AluOpType
    DR = mybir.MatmulPerfMode.DoubleRow

    nc = bacc.Bacc(trn_type="TRN2", target_bir_lowering=False, debug=False)

    # ---- DRAM tensors ----
    d_plan = nc.dram_tensor("planTg", [t_steps, 4, bl], f32r, kind="ExternalInput").ap()
    d_st0 = nc.dram_tensor("state0T", [3, bl], f32r, kind="ExternalInput").ap()
    d_c0c = nc.dram_tensor("c0cT", [D, bl], bf16, kind="ExternalInput").ap()
    d_p1 = nc.dram_tensor("p1T", [FF, bl], fp8, kind="ExternalInput").ap()
    d_w7cs = nc.dram_tensor("w7cs", [128, D], f32r, kind="ExternalInput").ap()
    d_u1 = nc.dram_tensor("u1t", [128, FF], f32r, kind="ExternalInput").ap()
    d_w2 = nc.dram_tensor("w2s8", [128, KF, D], fp8, kind="ExternalInput").ap()
    d_wd1 = nc.dram_tensor("wd1t", [128, KD, HID], bf16, kind="ExternalInput").ap()
    d_wd2 = nc.dram_tensor("wd2t", [HID, 3], bf16, kind="ExternalInput").ap()
    d_rdneg = nc.dram_tensor("rdnegr", [HID, HID], bf16, kind="ExternalInput").ap()
    d_bd2 = nc.dram_tensor("bd2v", [3, 1], f32, kind="ExternalInput").ap()
    d_ones = nc.dram_tensor("ones64", [128, HID], bf16, kind="ExternalInput").ap()
    d_eye = nc.dram_tensor("eye128", [128, 128], fp8, kind="ExternalInput").ap()
    d_zero = nc.dram_tensor("zeros121", [121, bl], f32r, kind="ExternalInput").ap()
    d_out = nc.dram_tensor("outT", [t_steps, 3, bl], f32r, kind="ExternalOutput").ap()

    with tile.TileContext(nc) as tc, ExitStack() as ctx:
        wp = ctx.enter_context(tc.tile_pool(name="w", bufs=1))

        def wtile(name, shape, src, dt_):
            t_ = wp.tile(shape, dt_, tag=name, name=name)
            nc.sync.dma_start(t_[:], src)
            return t_

        w7cs = wtile("w7cs", [128, D], d_w7cs[:, :], f32r)
        u1t = wtile("u1t", [128, FF], d_u1[:, :], f32r)
        w2t = wtile("w2t", [128, KF, D], d_w2[:, :, :], fp8)
        wd1t = wtile("wd1t", [128, KD, HID], d_wd1[:, :, :], bf16)
        wd2t = wtile("wd2t", [HID, 3], d_wd2[:, :], bf16)
        rdnegr = wtile("rdnegr", [HID, HID], d_rdneg[:, :], bf16)
        bd2v = wtile("bd2v", [3, 1], d_bd2[:, :], f32)
        ones64 = wtile("ones64", [128, HID], d_ones[:, :], bf16)
        eye128 = wtile("eye128", [128, 128], d_eye[:, :], fp8)
        epsb = wp.tile([HID, 1], f32, tag="epsb", name="epsb")
        nc.vector.memset(epsb[:], LN_EPS)

        # per-group persistent activation tiles (in-place rewrite per step)
        c0c_g, p1_g, h8_g, w3_g = [], [], [], []
        for n in range(NT):
            cs = slice(n * TN, (n + 1) * TN)
            c = wp.tile([128, KD, TN], bf16, tag=f"c0c{n}", name=f"c0c{n}")
            for k in range(KD):
                nc.sync.dma_start(c[:, k, :], d_c0c[k * 128:(k + 1) * 128, cs])
            c0c_g.append(c)
            p = wp.tile([128, KF, TN], fp8, tag=f"p1{n}", name=f"p1{n}")
            for q in range(KF):
                nc.sync.dma_start(p[:, q, :], d_p1[q * 128:(q + 1) * 128, cs])
            p1_g.append(p)
            h8_g.append(wp.tile([128, KF, TN], fp8, tag=f"h8{n}", name=f"h8{n}"))
            w3_g.append(wp.tile([128, KD, TN], bf16, tag=f"w3{n}", name=f"w3{n}"))

        # state+plan tiles: z7[t] rows 0..2 = state_t, rows 3..6 = [plan_t*gate; gate]
        zp = ctx.enter_context(tc.tile_pool(name="zp", bufs=2))
        z7s = [zp.tile([128, bl], f32r, tag="z7", name=f"z7_{t}") for t in range(t_steps + 1)]
        # rows 7:128 are a K-pad read by the (zero-padded) U1/W7cs matmuls;
        # zero both physical ring slots once so they never contain Inf/NaN bits
        nc.sync.dma_start(z7s[0][7:128, :], d_zero[:, :])
        nc.sync.dma_start(z7s[1][7:128, :], d_zero[:, :])
        nc.sync.dma_start(z7s[0][0:3, :], d_st0[:, :])
        for t in range(t_steps):
            nc.sync.dma_start(z7s[t][3:7, :], d_plan[t, :, :])

        # working pools
        sp = ctx.enter_context(tc.tile_pool(name="sp", bufs=2))
        pp = ctx.enter_context(tc.tile_pool(name="pp", bufs=8, space="PSUM"))

        def w1_phase(t, n):
            # h1 = relu(U1@z7 + P1) -> fp8 (scaled by S1)
            cs = slice(n * TN, (n + 1) * TN)
            h8 = h8_g[n]
            for q in range(KF):
                qs = slice(q * 128, (q + 1) * 128)
                ps = pp.tile([128, TN], f32, tag="ps", name="ps")
                nc.tensor.matmul(ps[:], u1t[:, qs], z7s[t][:, cs],
                                 start=True, stop=False)
                nc.tensor.matmul(ps[:], eye128[:, :], p1_g[n][:, q, :],
                                 start=False, stop=True)
                if q % 2 == 0:
                    nc.scalar.activation(h8[:, q, :], ps[:], AF.Relu)
                else:
                    nc.vector.tensor_scalar(h8[:, q, :], ps[:], 0.0, None, OP.max)

        def w2_pair(t, na, nb):
            # w = (W7cs@z7 + W2s@h1)/(S1*S2) + C0c -> bf16   (low-rank residual
            # term re-accumulated in f32 psum: no fp8 noise on the residual).
            # Two groups share each DoubleRow weight load: DR disables the
            # LDWEIGHTS/MATMUL overlap, so back-to-back same-weight matmuls
            # amortize the serialized 256-column weight load.
            csa = slice(na * TN, (na + 1) * TN)
            csb = slice(nb * TN, (nb + 1) * TN)
            for m in range(KD):
                ms = slice(m * 128, (m + 1) * 128)
                psa = pp.tile([128, TN], f32, tag="ps", name="ps")
                psb = pp.tile([128, TN], f32, tag="ps", name="ps")
                nc.tensor.matmul(psa[:], w7cs[:, ms], z7s[t][:, csa],
                                 start=True, stop=False)
                nc.tensor.matmul(psb[:], w7cs[:, ms], z7s[t][:, csb],
                                 start=True, stop=False)
                for kk in range(KF // 2):
                    wsl = w2t[:, 2 * kk:2 * kk + 2, ms]
                    nc.tensor.matmul(psa[:], wsl, h8_g[na][:, 2 * kk:2 * kk + 2, :],
                                     start=False, stop=(kk == KF // 2 - 1),
                                     perf_mode=DR)
                    nc.tensor.matmul(psb[:], wsl, h8_g[nb][:, 2 * kk:2 * kk + 2, :],
                                     start=False, stop=(kk == KF // 2 - 1),
                                     perf_mode=DR)
                nc.vector.scalar_tensor_tensor(w3_g[na][:, m, :], psa[:], 1.0 / (S1 * S2),
                                               c0c_g[na][:, m, :], OP.mult, OP.add)
                nc.vector.scalar_tensor_tensor(w3_g[nb][:, m, :], psb[:], 1.0 / (S1 * S2),
                                               c0c_g[nb][:, m, :], OP.mult, OP.add)

        def stats_phase(t, n):
            w3 = w3_g[n]
            # LN2 stats: m2 = sum(w)/D, var = sum(w^2)/D - m2^2
            mps = pp.tile([HID, TN], f32, tag="ps", name="ps")
            for k in range(KD):
                nc.tensor.matmul(mps[:], ones64[:, :], w3[:, k, :],
                                 start=(k == 0), stop=(k == KD - 1))
            m2sb = sp.tile([HID, TN], bf16, tag="m2", name="m2")
            nc.scalar.activation(m2sb[:], mps[:], AF.Copy, scale=1.0 / D)

            # dps = Wd1g@w - m2*rd, with the m2*rd correction folded in as a
            # K=1 matmul (rdneg stationary, m2 row moving)
            dps = pp.tile([HID, TN], f32, tag="ps", name="ps")
            for k in range(KD):
                nc.tensor.matmul(dps[:], wd1t[:, k, :], w3[:, k, :],
                                 start=(k == 0), stop=False)
            nc.tensor.matmul(dps[:], rdnegr[:, :], m2sb[:, :],
                             start=False, stop=True)

            wsq = sp.tile([128, KD, TN], bf16, tag="wsq", name="wsq")
            nc.scalar.activation(wsq[:, :, :], w3[:, :, :], AF.Square)
            eps2 = pp.tile([HID, TN], f32, tag="ps", name="ps")
            for k in range(KD):
                nc.tensor.matmul(eps2[:], ones64[:, :], wsq[:, k, :],
                                 start=(k == 0), stop=(k == KD - 1))
            ew2 = sp.tile([HID, TN], bf16, tag="ew2", name="ew2")
            nc.scalar.activation(ew2[:], eps2[:], AF.Copy, scale=1.0 / D)

            # inv2 = rsqrt(var+eps);  pre1 = dps * inv2
            msq = sp.tile([HID, TN], bf16, tag="msq", name="msq")
            nc.vector.tensor_tensor(msq[:], m2sb[:], m2sb[:], OP.mult)
            varb = sp.tile([HID, TN], bf16, tag="varb", name="varb")
            nc.vector.tensor_tensor(varb[:], ew2[:], msq[:], OP.subtract)
            lnv = sp.tile([HID, TN], bf16, tag="lnv", name="lnv")
            nc.scalar.activation(lnv[:], varb[:], AF.Ln, bias=epsb[:])
            inv2 = sp.tile([HID, TN], bf16, tag="inv2", name="inv2")
            nc.scalar.activation(inv2[:], lnv[:], AF.Exp, scale=-0.5)
            pre1 = sp.tile([HID, TN], bf16, tag="pre1", name="pre1")
            nc.vector.tensor_tensor(pre1[:], dps[:], inv2[:], OP.mult)

            # elu = max(exp(min(x,0))-1, x)
            emin = sp.tile([HID, TN], bf16, tag="emin", name="emin")
            nc.vector.tensor_scalar(emin[:], pre1[:], 0.0, None, OP.min)
            eexp = sp.tile([HID, TN], bf16, tag="eexp", name="eexp")
            nc.scalar.activation(eexp[:], emin[:], AF.Exp)
            el = sp.tile([HID, TN], bf16, tag="el", name="el", bufs=NT)
            nc.vector.scalar_tensor_tensor(el[:], eexp[:], 1.0, pre1[:],
                                           OP.subtract, OP.max)
            return el

        def tail_phase(t, n, el):
            # upd = Wd2 @ elu + bd2 ; state_{t+1} = state_t + upd
            cs = slice(n * TN, (n + 1) * TN)
            d2 = pp.tile([3, TN], f32, tag="ps", name="ps")
            nc.tensor.matmul(d2[:], wd2t[:, :], el[:], start=True, stop=True)
            nc.vector.scalar_tensor_tensor(z7s[t + 1][0:3, cs], d2[:], bd2v[:],
                                           z7s[t][0:3, cs], OP.add, OP.add)

        # 4-stage software pipeline over global slots s = t*NT + n, skewed by
        # one group per stage: W1(s) || W2(s-1) || stats+normalize(s-2) ||
        # decoder-tail(s-3). Slots cross step boundaries (state of group g is
        # ready 5 slots before step t+1 group g needs it). Keeps the PE
        # instruction stream free of long waits and HAM warm.
        els = {}
        S = t_steps * NT
        for s in range(S + 5):
            if s < S:
                w1_phase(s // NT, s % NT)
            if 2 <= s < S + 2 and (s - 2) % 2 == 0:
                w2_pair((s - 2) // NT, (s - 2) % NT, (s - 1) % NT)
            if 3 <= s < S + 3:
                els[s - 3] = stats_phase((s - 3) // NT, (s - 3) % NT)
            if 5 <= s < S + 5:
                t3, n3 = (s - 5) // NT, (s - 5) % NT
                tail_phase(t3, n3, els.pop(s - 5))
                if n3 == NT - 1:
                    nc.sync.dma_start(d_out[t3, :, :], z7s[t3 + 1][0:3, :])

    import concourse.bacc as bacc_mod
    if not getattr(bacc_mod, "_act_tables_patched", False):
        _orig_tables = bacc_mod.get_activation_tables
        _KEEP = "natural_log_exp_and_others"

        def _one_set_tables(arch):
            t = _orig_tables(arch)
            return {name: (fns if name == _KEEP else set()) for name, fns in t.items()}

        bacc_mod.get_activation_tables = _one_set_tables
        bacc_mod._act_tables_patched = True
    nc.compile()
    return nc


def _prep(inputs):
    """Host-side: fold the attention block, LN1, gains and biases into
    C0c/P1/W7c/U1; transpose weights to lhsT layouts; shard batch."""
    import ml_dtypes

    g = {k: np.asarray(v, dtype=np.float32) for k, v in inputs.items()}
    for zk in ("beta1", "b1", "b2", "beta2", "bd1"):
        assert np.max(np.abs(g[zk])) == 0.0, f"kernel assumes {zk} == 0"

    Wv = g["Wqkv"][2 * D:, :]
    bv = g["bqkv"][2 * D:]
    A = np.eye(D, dtype=np.float32) + g["Wo"] @ Wv           # [D, D]
    ab = g["Wo"] @ bv + g["bo"]                              # [D]

    iH = g["init_hidden"] + g["bs"][None, :]                 # [B, D]
    Cfull = iH @ A.T + ab[None, :]                           # [B, D] (host GEMM)
    C0cf = (Cfull - Cfull.mean(axis=1, keepdims=True)) * g["g1"][None, :]

    # W7 rows match z7 rows: [state (3); plan*gate (3); gate (1)]
    W7 = np.concatenate([(A @ g["Ws"]).T, (A @ g["Wp"]).T, (A @ g["bp"])[None, :]], 0)
    W7c = (W7 - W7.mean(axis=1, keepdims=True)) * g["g1"][None, :]  # [7, D]

    U1 = S1 * (g["W1"] @ W7c.T)                              # [FF, 7]
    P1 = S1 * (C0cf @ g["W1"].T)                             # [B, FF] (host GEMM)

    b16 = lambda a: np.ascontiguousarray(a).astype(ml_dtypes.bfloat16)
    f8 = lambda a: np.ascontiguousarray(np.clip(a, -240, 240)).astype(ml_dtypes.float8_e4m3)

    def lhsT_pack(w, kchunks):   # w: [out, in] -> [128, kchunks, out]
        return w.T.reshape(kchunks, 128, w.shape[0]).transpose(1, 0, 2)

    Wd1g = (g["Wd1"] * g["g2"][None, :]).astype(ml_dtypes.bfloat16).astype(np.float32)
    pad128 = lambda a: np.concatenate(
        [a, np.zeros((128 - a.shape[0], a.shape[1]), np.float32)], 0)
    shared = {
        "w7cs": pad128(np.ascontiguousarray(W7c * (S1 * S2))),
        "u1t": pad128(np.ascontiguousarray(U1.T)),
        "w2s8": f8(lhsT_pack(g["W2"] * S2, KF)),
        "wd1t": b16(lhsT_pack(Wd1g, KD)),
        "wd2t": b16(g["Wd2"].T),
        "rdnegr": b16(np.tile(-Wd1g.sum(axis=1).reshape(1, -1) / HID, (HID, 1))),
        "bd2v": np.ascontiguousarray(g["bd2"].reshape(-1, 1)),
        "ones64": np.ones((128, HID), dtype=ml_dtypes.bfloat16),
        "eye128": np.eye(128, dtype=np.float32).astype(ml_dtypes.float8_e4m3),
    }

    gate = g["gate"][:, 0]                                   # [B]
    pgate = g["plan"] * g["gate"][:, None, :]                # [B, T, 3]
    planT = pgate.transpose(1, 2, 0)                         # [T, 3, B]
    planTg = np.concatenate(
        [planT, np.broadcast_to(gate[None, None, :], (T, 1, B))], axis=1
    )                                                        # [T, 4, B]
    st0 = g["init_state"][:, :3].T                           # [3, B]
    c0cT = C0cf.T.astype(ml_dtypes.bfloat16)                 # [D, B]
    p1T = np.clip(P1.T, -240, 240).astype(ml_dtypes.float8_e4m3)  # [FF, B]

    in_maps = []
    for c in range(NCORES):
        cs = slice(c * BL, (c + 1) * BL)
        m = dict(shared)
        m["c0cT"] = np.ascontiguousarray(c0cT[:, cs])
        m["p1T"] = np.ascontiguousarray(p1T[:, cs])
        m["planTg"] = np.ascontiguousarray(planTg[:, :, cs])
        m["zeros121"] = np.zeros((121, BL), np.float32)
        m["state0T"] = np.ascontiguousarray(st0[:, cs])
        in_maps.append(m)
    return in_maps


def run(inputs, trace=False, trace_kwargs=None):
    from concourse.bass_utils import run_bass_kernel_spmd

    if "nc" not in _STATE:
        _STATE["nc"] = _build_nc()
    in_maps = _prep(inputs)
    res = run_bass_kernel_spmd(
        _STATE["nc"], in_maps, list(range(NCORES)), trace=trace,
        **(trace_kwargs or {}),
    )
    out = np.empty((B, T, 3), dtype=np.float32)
    for c in range(NCORES):
        outT = np.asarray(res.results[c]["outT"], dtype=np.float32)  # [T, 3, BL]
        out[c * BL:(c + 1) * BL] = outT.transpose(2, 0, 1)
    return out, res


def kernel(**inputs) -> np.ndarray:
    out, _ = run(inputs)
    return out
